# revision 20
# baseline (speedup 1.0000x reference)
"""BCE + weighted Dice loss on 8 Trainium2 NeuronCores.

Full inputs logits/targets [4,3,128,128,128] f32 are sharded along the depth
axis D=128 into 8 slices of 16 and converted to bf16 on the host. On the host
each (b,c) slab (16*128*128 elems) is reshaped to a [128, 2048] block and four
slabs are concatenated into a [128, 8192] "quad" tile, so per-slab device
reductions are contiguous 2048-column blocks (3 quads of 4 slabs = 12 slabs).

Math (s := sigmoid(-x)):
  sum(prob)    = N - sum(s)        bce_sum = -sum(ln s) - sum(x*t)
  sum(prob*t)  = sum(t) - sum(s*t)
  pred         = (x >= 0.5) == (s <= sigmoid(-0.5))   [bf16-monotone exact]

Engine split (measured costs per [128,8192] quad):
  ScalarE (the wall, ~47us busy): sigmoid (7.1us/quad, accum -> sum s),
      one table switch, then ln(s) per half-quad (accum -> sum ln s).
  VectorE: pred = s <= 0.37754 (4x mode, 2.3us/quad), scalar_tensor_tensor
      x*t accumulate per quarter-quad (accumulating DVE ops always run 1x),
      PSUM diag extraction (ident mask + X-reduce) for the matmul outputs.
  TensorE: per 128-col chunk, t_c is the stationary operand; streams:
      s_c -> global sum(s*t) diag bank; pred_c -> per-slab sum(t*pred) diag
      (4 rotating banks); pred_c stationary + ones column -> per-slab pred
      sums; t_c + ones column -> per-slab t sums (N=1 matmuls, groups seeded
      with a zero-data start=True matmul to dodge a start-drop race).

DMA: transfers fair-share bandwidth across everything outstanding, so
starts are staged with scheduler-time waits: x first (sigmoid-paced),
t quads interleaved behind (TensorE/STT-paced).

Device outputs per core: stats [128, 64] f32 partials; host sums partitions
and combines across cores.
"""

import sys

if "/opt/trn_rl_repo" not in sys.path:
    sys.path.insert(0, "/opt/trn_rl_repo")

import numpy as np

import concourse.bacc as bacc
import concourse.mybir as mybir
from concourse import tile
from concourse.alu_op_type import AluOpType
from concourse.bass_utils import run_bass_kernel_spmd

# Problem geometry (hardcoded per harness contract).
B, C, D, H, W = 4, 3, 128, 128, 128
N_CORES = 8
D_SHARD = D // N_CORES            # 16
SLABS = B * C                     # 12 (b,c) slabs per core
P = 128
F = 2048                          # slab block columns
QUADS = 3
QS = SLABS // QUADS               # 4 slabs per quad
QF = QS * F                       # 8192
N_TOTAL = B * C * D * H * W

# sigmoid(-0.5); pred = (x >= 0.5) <=> (s <= THR) for bf16 grids (verified)
THR = 0.37754067

# stats column map
SC_SIG = 0      # 4 sigmoid accums (q0a, q0b, q1, q2)
SC_LN = 4       # 6 ln accums (half-quads)
SC_TP = 10      # 12 per-slab sum(t*pred) diag reductions
SC_ST = 22      # global sum(s*t) diag reduction
SC_TCOL = 24    # 12 per-slab t column-sums (from PSUM)
SC_PCOL = 36    # 12 per-slab pred column-sums
SC_XT = 48      # 12 scalar_tensor_tensor x*t accums (quarter-quads)

_CACHED = {}


def _build():
    if "nc" in _CACHED:
        return _CACHED["nc"]
    AFT = mybir.ActivationFunctionType
    f32 = mybir.dt.float32
    bf16 = mybir.dt.bfloat16

    nc = bacc.Bacc("TRN2", target_bir_lowering=False, debug=False,
                   num_devices=N_CORES)
    x_d = nc.dram_tensor("logits", [QUADS, P, QF], bf16, kind="ExternalInput")
    t_d = nc.dram_tensor("targets", [QUADS, P, QF], bf16, kind="ExternalInput")
    id_d = nc.dram_tensor("ident", [P, 128], bf16, kind="ExternalInput")
    st_d = nc.dram_tensor("stats", [P, 64], f32, kind="ExternalOutput")

    with tile.TileContext(nc) as tc:
        with (
            tc.tile_pool(name="x", bufs=QUADS) as x_pool,
            tc.tile_pool(name="t", bufs=QUADS) as t_pool,
            tc.tile_pool(name="s", bufs=QUADS) as s_pool,
            tc.tile_pool(name="pred", bufs=QUADS) as pred_pool,
            tc.tile_pool(name="u", bufs=1) as u_pool,
            tc.tile_pool(name="u2", bufs=1) as u2_pool,
            tc.tile_pool(name="scr", bufs=2) as scr_pool,
            tc.tile_pool(name="misc", bufs=1) as misc_pool,
            tc.tile_pool(name="psum", bufs=1, space="PSUM") as psum_pool,
        ):
            stats = misc_pool.tile([P, 64], f32)
            nc.vector.memset(stats[:], 0.0)
            ones = misc_pool.tile([P, 1], bf16)
            nc.vector.memset(ones[:], 1.0)
            zcol = misc_pool.tile([P, 1], bf16)
            nc.vector.memset(zcol[:], 0.0)
            # tiny dummy sigmoid: hoists the sigmoid ACT_TABLE_LOAD to t=0
            # instead of paying it after the first x data arrives
            warm = misc_pool.tile([P, 1], bf16)
            nc.scalar.activation(warm[:], ones[:], AFT.Sigmoid, scale=-1.0)
            ident = misc_pool.tile([P, 128], bf16)
            nc.sync.dma_start(ident[:], id_d[:])

            p_st = psum_pool.tile([P, 128], f32, name="p_st", tag="p_st")
            p_tp = [psum_pool.tile([P, 128], f32, name=f"p_tp{i}",
                                   tag=f"p_tp{i}") for i in range(4)]
            p_cols = psum_pool.tile([P, 24], f32, name="p_cols", tag="p_cols")

            xq = [x_pool.tile([P, QF], bf16, tag="x", name=f"xq{q}")
                  for q in range(QUADS)]
            tq = [t_pool.tile([P, QF], bf16, tag="t", name=f"tq{q}")
                  for q in range(QUADS)]
            sq = [s_pool.tile([P, QF], bf16, tag="s", name=f"sq{q}")
                  for q in range(QUADS)]
            pq = [pred_pool.tile([P, QF], bf16, tag="pred", name=f"pq{q}")
                  for q in range(QUADS)]
            u = u_pool.tile([P, QF // 2], bf16)
            u2 = u2_pool.tile([P, QF // 4], bf16)

            # ---- DMA: transfers fair-share bandwidth across everything
            # outstanding, so stage starts with scheduler-time waits to get
            # x (sigmoid-paced) ahead of t (TensorE-paced). Big slices keep
            # per-partition descriptor lines >= 4KB.
            with tc.tile_wait_until(0.0):
                for k in range(4):
                    sl = slice(k * (QF // 4), (k + 1) * (QF // 4))
                    nc.sync.dma_start(xq[0][:, sl], x_d[0][:, sl])
            with tc.tile_wait_until(0.003):
                nc.sync.dma_start(xq[1][:], x_d[1])
            with tc.tile_wait_until(0.009):
                nc.sync.dma_start(tq[0][:], t_d[0])
            with tc.tile_wait_until(0.015):
                nc.sync.dma_start(xq[2][:], x_d[2])
            with tc.tile_wait_until(0.021):
                nc.sync.dma_start(tq[1][:], t_d[1])
            with tc.tile_wait_until(0.026):
                nc.sync.dma_start(tq[2][:], t_d[2])

            # ---- ScalarE phase 1: sigmoid (accum -> sum s) ----
            h = QF // 2
            nc.scalar.activation(sq[0][:, 0:h], xq[0][:, 0:h], AFT.Sigmoid,
                                 scale=-1.0, accum_out=stats[:, SC_SIG:SC_SIG + 1])
            nc.scalar.activation(sq[0][:, h:], xq[0][:, h:], AFT.Sigmoid,
                                 scale=-1.0, accum_out=stats[:, SC_SIG + 1:SC_SIG + 2])
            nc.scalar.activation(sq[1][:], xq[1][:], AFT.Sigmoid,
                                 scale=-1.0, accum_out=stats[:, SC_SIG + 2:SC_SIG + 3])
            nc.scalar.activation(sq[2][:], xq[2][:], AFT.Sigmoid,
                                 scale=-1.0, accum_out=stats[:, SC_SIG + 3:SC_SIG + 4])

            # ---- ScalarE phase 2: ln(s) per half-quad (accum -> sum ln s).
            # Held behind all sigmoids at schedule time so the two table
            # loads happen exactly once each (the scheduler dispatches by
            # readiness and would otherwise interleave sigma/ln).
            with tc.tile_wait_until(0.028):
                for q in range(QUADS):
                    for hh in range(2):
                        sl = slice(hh * h, (hh + 1) * h)
                        col = SC_LN + 2 * q + hh
                        nc.scalar.activation(u[:], sq[q][:, sl], AFT.Ln,
                                             accum_out=stats[:, col:col + 1])

            # ---- VectorE: pred (4x mode, issued up front) ----
            for q in range(QUADS):
                nc.vector.tensor_scalar(out=pq[q][:], in0=sq[q][:],
                                        scalar1=THR, scalar2=None,
                                        op0=AluOpType.is_le)

            # ---- TensorE: t-stationary chunks; DVE queue interleaves the
            # x*t STT (per quad, before that quad's slabs) with the per-slab
            # tp-diag extractions so neither blocks PSUM bank rotation ----
            for s_i in range(SLABS):
                q, j = divmod(s_i, QS)
                if j == 0:
                    qt = QF // 4
                    for hh in range(4):
                        sl = slice(hh * qt, (hh + 1) * qt)
                        col = SC_XT + 4 * q + hh
                        nc.vector.scalar_tensor_tensor(
                            out=u2[:], in0=xq[q][:, sl], scalar=1.0,
                            in1=tq[q][:, sl], op0=AluOpType.mult,
                            op1=AluOpType.mult,
                            accum_out=stats[:, col:col + 1])
                base = j * F
                tp_bank = p_tp[s_i % 4]
                # N=1 matmuls with start=True unreliably drop their output
                # (weight-load race); seed each per-slab N=1 group with a
                # zero-data start=True matmul so real chunks use start=False.
                seed_w = tq[q][:, base:base + 128]
                nc.tensor.matmul(p_cols[:, s_i:s_i + 1], seed_w, zcol[:],
                                 start=True, stop=False)
                nc.tensor.matmul(p_cols[:, 12 + s_i:13 + s_i], seed_w,
                                 zcol[:], start=True, stop=False)
                for c in range(F // 128):
                    sl = slice(base + c * 128, base + (c + 1) * 128)
                    first = c == 0
                    last = c == F // 128 - 1
                    tc_w = tq[q][:, sl]
                    nc.tensor.matmul(p_st[:, :], tc_w, sq[q][:, sl],
                                     start=(s_i == 0 and first),
                                     stop=(s_i == SLABS - 1 and last))
                    nc.tensor.matmul(tp_bank[:, :], tc_w, pq[q][:, sl],
                                     start=first, stop=last)
                    nc.tensor.matmul(p_cols[:, 12 + s_i:13 + s_i],
                                     pq[q][:, sl], ones[:],
                                     start=False, stop=last)
                    nc.tensor.matmul(p_cols[:, s_i:s_i + 1], tc_w, ones[:],
                                     start=False, stop=last)

                # extract per-slab sum(t*pred): diag mask then free-dim reduce
                mtp = scr_pool.tile([P, 128], f32, tag="mtp", name=f"mtp{s_i}")
                nc.vector.tensor_tensor(out=mtp[:], in0=tp_bank[:, :],
                                        in1=ident[:], op=AluOpType.mult)
                col = SC_TP + s_i
                nc.vector.tensor_reduce(out=stats[:, col:col + 1], in_=mtp[:],
                                        axis=mybir.AxisListType.X,
                                        op=AluOpType.add)

            # ---- Epilogue ----
            mst = scr_pool.tile([P, 128], f32, tag="mtp", name="mst")
            nc.vector.tensor_tensor(out=mst[:], in0=p_st[:, :], in1=ident[:],
                                    op=AluOpType.mult)
            nc.vector.tensor_reduce(out=stats[:, SC_ST:SC_ST + 1], in_=mst[:],
                                    axis=mybir.AxisListType.X,
                                    op=AluOpType.add)
            nc.vector.tensor_copy(stats[:, SC_TCOL:SC_TCOL + 24],
                                  p_cols[:, 0:24])
            nc.sync.dma_start(st_d[:], stats[:])

    nc.compile()
    _CACHED["nc"] = nc
    return nc


def _to_bf16_bits(a: np.ndarray) -> np.ndarray:
    """f32 -> bf16 bits with round-to-nearest-even, returned as uint16."""
    u = np.ascontiguousarray(a, dtype=np.float32).view(np.uint32)
    rounded = ((u + 0x7FFF + ((u >> 16) & 1)) >> 16).astype(np.uint16)
    return rounded


def _shard_inputs(logits: np.ndarray, targets: np.ndarray):
    import ml_dtypes

    bf = ml_dtypes.bfloat16
    xb = _to_bf16_bits(logits).view(bf)
    tb = _to_bf16_bits(targets).view(bf)
    eye = np.eye(P, 128, dtype=np.float32).astype(bf)
    in_maps = []
    for i in range(N_CORES):
        sl = slice(i * D_SHARD, (i + 1) * D_SHARD)
        # [B, C, 16, H, W] -> per-slab [128, 2048] blocks -> quads' columns
        x = np.ascontiguousarray(xb[:, :, sl]).reshape(SLABS, P, F)
        t = np.ascontiguousarray(tb[:, :, sl]).reshape(SLABS, P, F)
        x = x.reshape(QUADS, QS, P, F).transpose(0, 2, 1, 3).reshape(QUADS, P, QF)
        t = t.reshape(QUADS, QS, P, F).transpose(0, 2, 1, 3).reshape(QUADS, P, QF)
        in_maps.append({"logits": np.ascontiguousarray(x),
                        "targets": np.ascontiguousarray(t),
                        "ident": eye})
    return in_maps


def _combine(results):
    """Host-side reduction of per-core stats partials to the scalar loss."""
    EPS = 1e-9
    S_s = S_l = S_xt = S_st = 0.0
    S_tp = np.zeros(SLABS)
    S_t = np.zeros(SLABS)
    S_pred = np.zeros(SLABS)
    for r in results:
        st = r["stats"].astype(np.float64)
        S_s += st[:, SC_SIG:SC_SIG + 4].sum()
        S_l += st[:, SC_LN:SC_LN + 6].sum()
        S_xt += st[:, SC_XT:SC_XT + 12].sum()
        S_st += st[:, SC_ST].sum()
        S_tp += st[:, SC_TP:SC_TP + 12].sum(axis=0)
        S_t += st[:, SC_TCOL:SC_TCOL + 12].sum(axis=0)
        S_pred += st[:, SC_PCOL:SC_PCOL + 12].sum(axis=0)

    sum_prob = N_TOTAL - S_s
    sum_pt = S_t.sum() - S_st               # sum(prob * t)
    bce = (-S_l - S_xt) / N_TOTAL

    union = sum_prob + S_t.sum()
    dice_loss = 1.0 - (2.0 * sum_pt + EPS) / union

    score = np.where(
        (S_t == 0) & (S_pred == 0),
        np.ones_like(S_t),
        (2.0 * S_tp + EPS) / (S_t + S_pred),
    ).reshape(B, C)
    per_class = score.mean(axis=0)

    loss = (bce + dice_loss * 0.5 + per_class[0] * 0.2
            + per_class[1] * 0.1 + per_class[2] * 0.2)
    return np.float32(loss)


def kernel(logits: np.ndarray, targets: np.ndarray) -> np.ndarray:
    nc = _build()
    in_maps = _shard_inputs(np.asarray(logits), np.asarray(targets))
    res = run_bass_kernel_spmd(nc, in_maps, list(range(N_CORES)))
    return _combine(res.results)


# revision 21
# speedup vs baseline: 1.1652x; 1.1652x over previous
"""BCE + weighted Dice loss on 8 Trainium2 NeuronCores.

Full inputs logits/targets [4,3,128,128,128] f32 are sharded along the depth
axis D=128 into 8 slices of 16 and converted to bf16 on the host. On the host
each (b,c) slab (16*128*128 elems) is reshaped to a [128, 2048] block and four
slabs are concatenated into a [128, 8192] "quad" tile, so per-slab device
reductions are contiguous 2048-column blocks (3 quads of 4 slabs = 12 slabs).

Math (s := sigmoid(-x)):
  sum(prob)    = N - sum(s)        bce_sum = -sum(ln s) - sum(x*t)
  sum(prob*t)  = sum(t) - sum(s*t)
  pred         = (x >= 0.5) == (s <= sigmoid(-0.5))   [bf16-monotone exact]

Engine split (measured costs per [128,8192] quad):
  ScalarE (the wall, ~47us busy): sigmoid (7.1us/quad, accum -> sum s),
      one table switch, then ln(s) per half-quad (accum -> sum ln s).
  VectorE: pred = s <= 0.37754 (4x mode, 2.3us/quad), scalar_tensor_tensor
      x*t accumulate per quarter-quad (accumulating DVE ops always run 1x),
      PSUM diag extraction (ident mask + X-reduce) for the matmul outputs.
  TensorE: per 128-col chunk, t_c is the stationary operand; streams:
      s_c -> global sum(s*t) diag bank; pred_c -> per-slab sum(t*pred) diag
      (4 rotating banks); pred_c stationary + ones column -> per-slab pred
      sums; t_c + ones column -> per-slab t sums (N=1 matmuls, groups seeded
      with a zero-data start=True matmul to dodge a start-drop race).

DMA: transfers fair-share bandwidth across everything outstanding, so
starts are staged with scheduler-time waits: x first (sigmoid-paced),
t quads interleaved behind (TensorE/STT-paced).

Device outputs per core: stats [128, 64] f32 partials; host sums partitions
and combines across cores.
"""

import sys

if "/opt/trn_rl_repo" not in sys.path:
    sys.path.insert(0, "/opt/trn_rl_repo")

import numpy as np

import concourse.bacc as bacc
import concourse.mybir as mybir
from concourse import tile
from concourse.alu_op_type import AluOpType
from concourse.bass_utils import run_bass_kernel_spmd

# Problem geometry (hardcoded per harness contract).
B, C, D, H, W = 4, 3, 128, 128, 128
N_CORES = 8
D_SHARD = D // N_CORES            # 16
SLABS = B * C                     # 12 (b,c) slabs per core
P = 128
F = 2048                          # slab block columns
QUADS = 3
QS = SLABS // QUADS               # 4 slabs per quad
QF = QS * F                       # 8192
N_TOTAL = B * C * D * H * W

# sigmoid(-0.5); pred = (x >= 0.5) <=> (s <= THR) for bf16 grids (verified)
THR = 0.37754067

# stats column map
SC_SIG = 0      # 4 sigmoid accums (q0a, q0b, q1, q2)
SC_LN = 4       # 6 ln accums (half-quads)
SC_TP = 10      # 12 per-slab sum(t*pred) diag reductions
SC_ST = 22      # global sum(s*t) diag reduction
SC_TCOL = 24    # 12 per-slab t column-sums (from PSUM)
SC_PCOL = 36    # 12 per-slab pred column-sums
SC_XT = 48      # 12 scalar_tensor_tensor x*t accums (quarter-quads)

_CACHED = {}


def _build():
    if "nc" in _CACHED:
        return _CACHED["nc"]
    AFT = mybir.ActivationFunctionType
    f32 = mybir.dt.float32
    bf16 = mybir.dt.bfloat16
    fp8 = mybir.dt.float8e4

    nc = bacc.Bacc("TRN2", target_bir_lowering=False, debug=False,
                   num_devices=N_CORES)
    x_d = nc.dram_tensor("logits", [QUADS, P, QF], bf16, kind="ExternalInput")
    t_d = nc.dram_tensor("targets", [QUADS, P, QF], bf16, kind="ExternalInput")
    id_d = nc.dram_tensor("ident", [P, 128], bf16, kind="ExternalInput")
    st_d = nc.dram_tensor("stats", [P, 64], f32, kind="ExternalOutput")

    with tile.TileContext(nc) as tc:
        with (
            tc.tile_pool(name="x", bufs=QUADS) as x_pool,
            tc.tile_pool(name="t", bufs=QUADS) as t_pool,
            tc.tile_pool(name="s", bufs=QUADS) as s_pool,
            tc.tile_pool(name="pred", bufs=QUADS) as pred_pool,
            tc.tile_pool(name="u", bufs=1) as u_pool,
            tc.tile_pool(name="u2", bufs=1) as u2_pool,
            tc.tile_pool(name="scr", bufs=2) as scr_pool,
            tc.tile_pool(name="misc", bufs=1) as misc_pool,
            tc.tile_pool(name="psum", bufs=1, space="PSUM") as psum_pool,
        ):
            stats = misc_pool.tile([P, 64], f32)
            nc.vector.memset(stats[:], 0.0)
            ones = misc_pool.tile([P, 1], bf16)
            nc.vector.memset(ones[:], 1.0)
            zcol = misc_pool.tile([P, 1], bf16)
            nc.vector.memset(zcol[:], 0.0)
            # tiny dummy sigmoid: hoists the sigmoid ACT_TABLE_LOAD to t=0
            # instead of paying it after the first x data arrives
            warm = misc_pool.tile([P, 1], bf16)
            nc.scalar.activation(warm[:], ones[:], AFT.Sigmoid, scale=-1.0)
            ident = misc_pool.tile([P, 128], bf16)
            nc.sync.dma_start(ident[:], id_d[:])

            p_st = psum_pool.tile([P, 128], f32, name="p_st", tag="p_st")
            p_tp = [psum_pool.tile([P, 128], f32, name=f"p_tp{i}",
                                   tag=f"p_tp{i}") for i in range(4)]
            p_cols = psum_pool.tile([P, 24], f32, name="p_cols", tag="p_cols")

            xq = [x_pool.tile([P, QF], bf16, tag="x", name=f"xq{q}")
                  for q in range(QUADS)]
            tq = [t_pool.tile([P, QF], bf16, tag="t", name=f"tq{q}")
                  for q in range(QUADS)]
            sq = [s_pool.tile([P, QF], bf16, tag="s", name=f"sq{q}")
                  for q in range(QUADS)]
            pq = [pred_pool.tile([P, QF], bf16, tag="pred", name=f"pq{q}")
                  for q in range(QUADS)]
            # scratch for discarded ln/STT outputs: fp8 halves the SBUF
            # write traffic (accumulators are fp32-internal, unaffected)
            u = u_pool.tile([P, QF // 2], fp8)
            u2 = u2_pool.tile([P, QF // 4], fp8)

            # ---- DMA: transfers fair-share bandwidth across everything
            # outstanding, so stage starts with scheduler-time waits to get
            # x (sigmoid-paced) ahead of t (TensorE-paced). Big slices keep
            # per-partition descriptor lines >= 4KB.
            with tc.tile_wait_until(0.0):
                for k in range(4):
                    sl = slice(k * (QF // 4), (k + 1) * (QF // 4))
                    nc.sync.dma_start(xq[0][:, sl], x_d[0][:, sl])
            with tc.tile_wait_until(0.003):
                nc.sync.dma_start(xq[1][:], x_d[1])
            with tc.tile_wait_until(0.010):
                nc.sync.dma_start(tq[0][:], t_d[0])
            with tc.tile_wait_until(0.013):
                nc.sync.dma_start(xq[2][:], x_d[2])
            with tc.tile_wait_until(0.021):
                nc.sync.dma_start(tq[1][:], t_d[1])
            with tc.tile_wait_until(0.026):
                nc.sync.dma_start(tq[2][:], t_d[2])

            # ---- ScalarE phase 1: sigmoid (accum -> sum s) ----
            h = QF // 2
            nc.scalar.activation(sq[0][:, 0:h], xq[0][:, 0:h], AFT.Sigmoid,
                                 scale=-1.0, accum_out=stats[:, SC_SIG:SC_SIG + 1])
            nc.scalar.activation(sq[0][:, h:], xq[0][:, h:], AFT.Sigmoid,
                                 scale=-1.0, accum_out=stats[:, SC_SIG + 1:SC_SIG + 2])
            nc.scalar.activation(sq[1][:], xq[1][:], AFT.Sigmoid,
                                 scale=-1.0, accum_out=stats[:, SC_SIG + 2:SC_SIG + 3])
            nc.scalar.activation(sq[2][:], xq[2][:], AFT.Sigmoid,
                                 scale=-1.0, accum_out=stats[:, SC_SIG + 3:SC_SIG + 4])

            # ---- ScalarE phase 2: ln(s) per half-quad (accum -> sum ln s).
            # Held behind all sigmoids at schedule time so the two table
            # loads happen exactly once each (the scheduler dispatches by
            # readiness and would otherwise interleave sigma/ln).
            with tc.tile_wait_until(0.028):
                for q in range(QUADS):
                    for hh in range(2):
                        sl = slice(hh * h, (hh + 1) * h)
                        col = SC_LN + 2 * q + hh
                        nc.scalar.activation(u[:], sq[q][:, sl], AFT.Ln,
                                             accum_out=stats[:, col:col + 1])

            # ---- VectorE: pred (4x mode, issued up front) ----
            for q in range(QUADS):
                nc.vector.tensor_scalar(out=pq[q][:], in0=sq[q][:],
                                        scalar1=THR, scalar2=None,
                                        op0=AluOpType.is_le)

            # ---- TensorE: t-stationary chunks; DVE queue interleaves the
            # x*t STT (per quad, before that quad's slabs) with the per-slab
            # tp-diag extractions so neither blocks PSUM bank rotation ----
            for s_i in range(SLABS):
                q, j = divmod(s_i, QS)
                if j == 0:
                    qt = QF // 4
                    for hh in range(4):
                        sl = slice(hh * qt, (hh + 1) * qt)
                        col = SC_XT + 4 * q + hh
                        nc.vector.scalar_tensor_tensor(
                            out=u2[:], in0=xq[q][:, sl], scalar=1.0,
                            in1=tq[q][:, sl], op0=AluOpType.mult,
                            op1=AluOpType.mult,
                            accum_out=stats[:, col:col + 1])
                base = j * F
                tp_bank = p_tp[s_i % 4]
                # N=1 matmuls with start=True unreliably drop their output
                # (weight-load race); seed each per-slab N=1 group with a
                # zero-data start=True matmul so real chunks use start=False.
                seed_w = tq[q][:, base:base + 128]
                nc.tensor.matmul(p_cols[:, s_i:s_i + 1], seed_w, zcol[:],
                                 start=True, stop=False)
                nc.tensor.matmul(p_cols[:, 12 + s_i:13 + s_i], seed_w,
                                 zcol[:], start=True, stop=False)
                for c in range(F // 128):
                    sl = slice(base + c * 128, base + (c + 1) * 128)
                    first = c == 0
                    last = c == F // 128 - 1
                    tc_w = tq[q][:, sl]
                    nc.tensor.matmul(p_st[:, :], tc_w, sq[q][:, sl],
                                     start=(s_i == 0 and first),
                                     stop=(s_i == SLABS - 1 and last))
                    nc.tensor.matmul(tp_bank[:, :], tc_w, pq[q][:, sl],
                                     start=first, stop=last)
                    nc.tensor.matmul(p_cols[:, 12 + s_i:13 + s_i],
                                     pq[q][:, sl], ones[:],
                                     start=False, stop=last)
                    nc.tensor.matmul(p_cols[:, s_i:s_i + 1], tc_w, ones[:],
                                     start=False, stop=last)

                # extract per-slab sum(t*pred): diag mask then free-dim reduce
                mtp = scr_pool.tile([P, 128], f32, tag="mtp", name=f"mtp{s_i}")
                nc.vector.tensor_tensor(out=mtp[:], in0=tp_bank[:, :],
                                        in1=ident[:], op=AluOpType.mult)
                col = SC_TP + s_i
                nc.vector.tensor_reduce(out=stats[:, col:col + 1], in_=mtp[:],
                                        axis=mybir.AxisListType.X,
                                        op=AluOpType.add)

            # ---- Epilogue ----
            mst = scr_pool.tile([P, 128], f32, tag="mtp", name="mst")
            nc.vector.tensor_tensor(out=mst[:], in0=p_st[:, :], in1=ident[:],
                                    op=AluOpType.mult)
            nc.vector.tensor_reduce(out=stats[:, SC_ST:SC_ST + 1], in_=mst[:],
                                    axis=mybir.AxisListType.X,
                                    op=AluOpType.add)
            nc.vector.tensor_copy(stats[:, SC_TCOL:SC_TCOL + 24],
                                  p_cols[:, 0:24])
            nc.sync.dma_start(st_d[:], stats[:])

    nc.compile()
    _CACHED["nc"] = nc
    return nc


def _to_bf16_bits(a: np.ndarray) -> np.ndarray:
    """f32 -> bf16 bits with round-to-nearest-even, returned as uint16."""
    u = np.ascontiguousarray(a, dtype=np.float32).view(np.uint32)
    rounded = ((u + 0x7FFF + ((u >> 16) & 1)) >> 16).astype(np.uint16)
    return rounded


def _shard_inputs(logits: np.ndarray, targets: np.ndarray):
    import ml_dtypes

    bf = ml_dtypes.bfloat16
    xb = _to_bf16_bits(logits).view(bf)
    tb = _to_bf16_bits(targets).view(bf)
    eye = np.eye(P, 128, dtype=np.float32).astype(bf)
    in_maps = []
    for i in range(N_CORES):
        sl = slice(i * D_SHARD, (i + 1) * D_SHARD)
        # [B, C, 16, H, W] -> per-slab [128, 2048] blocks -> quads' columns
        x = np.ascontiguousarray(xb[:, :, sl]).reshape(SLABS, P, F)
        t = np.ascontiguousarray(tb[:, :, sl]).reshape(SLABS, P, F)
        x = x.reshape(QUADS, QS, P, F).transpose(0, 2, 1, 3).reshape(QUADS, P, QF)
        t = t.reshape(QUADS, QS, P, F).transpose(0, 2, 1, 3).reshape(QUADS, P, QF)
        in_maps.append({"logits": np.ascontiguousarray(x),
                        "targets": np.ascontiguousarray(t),
                        "ident": eye})
    return in_maps


def _combine(results):
    """Host-side reduction of per-core stats partials to the scalar loss."""
    EPS = 1e-9
    S_s = S_l = S_xt = S_st = 0.0
    S_tp = np.zeros(SLABS)
    S_t = np.zeros(SLABS)
    S_pred = np.zeros(SLABS)
    for r in results:
        st = r["stats"].astype(np.float64)
        S_s += st[:, SC_SIG:SC_SIG + 4].sum()
        S_l += st[:, SC_LN:SC_LN + 6].sum()
        S_xt += st[:, SC_XT:SC_XT + 12].sum()
        S_st += st[:, SC_ST].sum()
        S_tp += st[:, SC_TP:SC_TP + 12].sum(axis=0)
        S_t += st[:, SC_TCOL:SC_TCOL + 12].sum(axis=0)
        S_pred += st[:, SC_PCOL:SC_PCOL + 12].sum(axis=0)

    sum_prob = N_TOTAL - S_s
    sum_pt = S_t.sum() - S_st               # sum(prob * t)
    bce = (-S_l - S_xt) / N_TOTAL

    union = sum_prob + S_t.sum()
    dice_loss = 1.0 - (2.0 * sum_pt + EPS) / union

    score = np.where(
        (S_t == 0) & (S_pred == 0),
        np.ones_like(S_t),
        (2.0 * S_tp + EPS) / (S_t + S_pred),
    ).reshape(B, C)
    per_class = score.mean(axis=0)

    loss = (bce + dice_loss * 0.5 + per_class[0] * 0.2
            + per_class[1] * 0.1 + per_class[2] * 0.2)
    return np.float32(loss)


def kernel(logits: np.ndarray, targets: np.ndarray) -> np.ndarray:
    nc = _build()
    in_maps = _shard_inputs(np.asarray(logits), np.asarray(targets))
    res = run_bass_kernel_spmd(nc, in_maps, list(range(N_CORES)))
    return _combine(res.results)
